# revision 39
# baseline (speedup 1.0000x reference)
"""Trainium2 Bass kernel for nn_AttentionPoolingTemporalEncoder.

Strategy (data-parallel over batch, 8 cores, 4 batch rows each):
  device:  h = relu(x @ Wp)               (fp8e4 DoubleRow matmuls, 2x PE rate;
                                           Wp pre-scaled x32 into e4m3 range)
           scores = h @ ((Wk @ qh)/sqrt(D))   (bk shifts cancel in softmax)
           p = exp(scores + maskbias)     (no running max; scores are O(5))
           U[h,:] = sum_s p[s,h] * h[s,:] ; Z[h] = sum_s p[s,h]
  host:    pooled = (U/Z) @ Wv (+bv) per head; @Wo+bo; @W2+b2; LayerNorm.
"""

import sys
import threading

import numpy as np

sys.path.insert(0, "/opt/trn_rl_repo")

from contextlib import ExitStack

import concourse.tile as tile
from concourse import bacc, mybir
from concourse.bass_utils import run_bass_kernel_spmd
from concourse.masks import make_identity


def _ensure_axon_ntff_hook_module():
    """Some images lack ``antenv.axon_hooks``; concourse imports it
    unconditionally when tracing is requested (e.g. via BASS_TRACE).
    Provide a minimal stand-in so that path degrades to no-trace
    instead of crashing."""
    try:
        from antenv import axon_hooks  # noqa: F401

        return
    except ImportError:
        pass
    import types

    mod = types.ModuleType("antenv.axon_hooks")
    mod._hook = None

    def set_axon_ntff_profile_hook(h):
        mod._hook = h

    def get_axon_ntff_profile_hook():
        return mod._hook

    mod.set_axon_ntff_profile_hook = set_axon_ntff_profile_hook
    mod.get_axon_ntff_profile_hook = get_axon_ntff_profile_hook
    sys.modules["antenv.axon_hooks"] = mod
    try:
        import antenv

        antenv.axon_hooks = mod
    except ImportError:
        pass


_ensure_axon_ntff_hook_module()

# Problem sizes (hardcoded per spec)
B, S, IN_DIM, E, H = 32, 4096, 1024, 512, 8
D = E // H
NCORES = 8
P = 128

_nc_cache = {}
_nc_lock = threading.Lock()


def build_nc(BL=B // NCORES, S_=S, I_=IN_DIM, has_bp=False, no_mask=False, trace_label=""):
    """Build + compile the per-core Bass program.

    BL: batch rows per core. S_: sequence length. I_: input dim.
    has_bp: emit the extra K=1 matmul adding the input-projection bias.
    """
    key = (BL, S_, I_, has_bp, no_mask)
    with _nc_lock:
        if key in _nc_cache:
            return _nc_cache[key]

    IC = I_ // P        # input-dim chunks
    EC = E // P         # embed-dim chunks
    S_TILES = S_ // P   # sequence tiles per batch row
    S_BLK = min(1024, S_)
    BLKS = S_ // S_BLK
    TPB = S_BLK // P    # s-tiles per DMA block

    f32 = mybir.dt.float32
    bf16 = mybir.dt.bfloat16
    fp8 = mybir.dt.float8e4
    DR = mybir.MatmulPerfMode.DoubleRow
    RELU = mybir.ActivationFunctionType.Relu
    EXP = mybir.ActivationFunctionType.Exp
    COPY = mybir.ActivationFunctionType.Copy

    nc = bacc.Bacc(
        "TRN2",
        target_bir_lowering=False,
        debug=False,
        enable_asserts=False,
        num_devices=NCORES,
    )

    # DRAM I/O (per-core shapes). The big matmul operands are fp8e4
    # (host-cast): quarters HBM traffic for x and runs the PE at 2x rate
    # via DoubleRow (contraction pairs of 128-chunks).
    # x layout: [b, s_chunk, quarter, partition, 2, s] so each quarter-load
    # is one contiguous 2KB run per partition (cheap HWDGE descriptors).
    NCH_ = S_ // min(1024, S_)
    SC_ = min(1024, S_)
    xt = nc.dram_tensor(
        "xt", [BL, NCH_, IC // 2, P, 2, SC_], fp8, kind="ExternalInput"
    ).ap()
    wp = nc.dram_tensor("wp", [IC, P, E], fp8, kind="ExternalInput").ap()
    wkq = nc.dram_tensor("wkq", [EC, P, H], bf16, kind="ExternalInput").ap()
    mb = nc.dram_tensor("mb", [BL, P, S_TILES], f32, kind="ExternalInput").ap()
    if has_bp:
        bp_d = nc.dram_tensor("bp", [1, E], bf16, kind="ExternalInput").ap()
    u_out = nc.dram_tensor("u_out", [BL, H, E], f32, kind="ExternalOutput").ap()
    z_out = nc.dram_tensor("z_out", [BL, H, 1], f32, kind="ExternalOutput").ap()

    with tile.TileContext(nc) as tc, ExitStack() as ctx:
        const = ctx.enter_context(tc.tile_pool(name="const", bufs=1))
        xp = ctx.enter_context(tc.tile_pool(name="xp", bufs=4))
        # h_se and p_sb stay resident for a whole batch row (the U/Z
        # accumulation runs as an end-of-row burst), plus slack so the next
        # row can start while the burst drains.
        hp = ctx.enter_context(tc.tile_pool(name="hp", bufs=S_TILES + 7))
        htp = ctx.enter_context(tc.tile_pool(name="htp", bufs=S_TILES + 6))
        pp = ctx.enter_context(tc.tile_pool(name="pp", bufs=8))
        scp = ctx.enter_context(tc.tile_pool(name="scp", bufs=6))
        mbp = ctx.enter_context(tc.tile_pool(name="mbp", bufs=2))
        uzp = ctx.enter_context(tc.tile_pool(name="uzp", bufs=2))
        ps_h = ctx.enter_context(tc.tile_pool(name="ps_h", bufs=3, space="PSUM"))
        ps_s = ctx.enter_context(tc.tile_pool(name="ps_s", bufs=2, space="PSUM"))
        ps_u = ctx.enter_context(tc.tile_pool(name="ps_u", bufs=2, space="PSUM"))
        ps_z = ctx.enter_context(tc.tile_pool(name="ps_z", bufs=1, space="PSUM"))

        # Resident constants
        wp_sb = const.tile([P, IC, E], fp8)
        nc.sync.dma_start(wp_sb[:], wp.rearrange("c p e -> p c e"))
        wkq_sb = const.tile([P, EC, H], bf16)
        nc.sync.dma_start(wkq_sb[:], wkq.rearrange("c p h -> p c h"))
        ones_t = const.tile([P, 2], bf16)
        nc.gpsimd.memset(ones_t[:], 1.0)
        if has_bp:
            ones_row = const.tile([1, P], bf16)
            nc.gpsimd.memset(ones_row[:], 1.0)
            bp_sb = const.tile([1, E], bf16)
            nc.sync.dma_start(bp_sb[:], bp_d[:])

        # Chunked x prefetch (1024 s = 8 tiles per chunk), issued ahead so
        # loads never queue behind the per-tile transposes.
        SC = min(1024, S_)
        NCH = S_ // SC
        chunks = [(bb, cc) for bb in range(BL) for cc in range(NCH)]

        def alloc_chunk():
            xt_c = xp.tile([P, IC, SC], fp8, tag="xchunk")
            return xt_c

        def load_chunk_part(xt_c, idx, q, eng=None):
            # Quarter-chunk loads (2 of 8 i-chunks, 256 KB) keep DMA bursts
            # short so the latency-critical transposes are never stuck
            # behind megabyte transfers. The host layout makes each quarter
            # one contiguous 2KB run per partition, so the HWDGE trigger is
            # cheap on the issuing engine.
            bb, cc = chunks[idx]
            (eng or nc.scalar).dma_start(xt_c[:, 2 * q : 2 * q + 2, :], xt[bb, cc, q])

        def load_chunk(idx):
            xt_c = alloc_chunk()
            for q in range(4):
                load_chunk_part(xt_c, idx, q)
            return xt_c

        # distance-3 prefetch: three chunks in flight ahead of the consumer
        bufq = [load_chunk(i) for i in range(min(3, len(chunks)))]
        chunk_idx = len(bufq) - 1
        part_sched = []  # [(tile_countdown, xt_c, idx, q), ...]

        # Two-phase rows: phase 1 is a pure, dense DoubleRow h-MM stream
        # (only x-prefetch deps -> HAM warms and stays warm); RELUs and
        # transposes run alongside and everything is kept resident. Phase 2
        # then does scores/EXP/U/Z, by which time the transposes are done.
        for b in range(BL):
            mb_t = mbp.tile([P, S_TILES], f32)
            nc.gpsimd.dma_start(mb_t[:], mb[b])
            u_ps = ps_u.tile([H, E], f32)
            z_ps = ps_z.tile([H, 2], f32)

            tiles = []  # [(t, h_se, ht_sb), ...]

            # ---- phase 1: h = relu(x @ Wp) for all 32 tiles ----
            for t in range(S_TILES):
                    TPC = SC // P
                    if t % TPC == 0:
                        # consume the next chunk; schedule the +3 chunk's
                        # quarter-loads spread over this chunk's tiles
                        x_sb = bufq.pop(0)
                        if chunk_idx + 1 < len(chunks):
                            chunk_idx += 1
                            nxt = alloc_chunk()
                            bufq.append(nxt)
                            for q in range(4):
                                part_sched.append((2 * q, nxt, chunk_idx, q))
                    # fire due quarter-loads, age the rest
                    still = []
                    for cd, xc, idx, q in part_sched:
                        if cd <= 0:
                            load_chunk_part(xc, idx, q)
                        else:
                            still.append((cd - 1, xc, idx, q))
                    part_sched[:] = still
                    # DoubleRow fp8: each MM contracts a pair of 128-deep
                    # i-chunks (256 total) at 2 fp8/cyc.
                    h_ps = ps_h.tile([P, E], f32)
                    for c in range(0, IC, 2):
                        nc.tensor.matmul(
                            h_ps[:],
                            x_sb[:, c : c + 2, (t % TPC) * P : (t % TPC + 1) * P],
                            wp_sb[:, c : c + 2, :],
                            start=(c == 0),
                            stop=(c == IC - 2) and not has_bp,
                            perf_mode=DR,
                        )
                    if has_bp:
                        nc.tensor.matmul(
                            h_ps[:],
                            ones_row[:],
                            bp_sb[:],
                            start=False,
                            stop=True,
                        )
                    h_se = hp.tile([P, E], bf16)
                    nc.scalar.activation(h_se[:], h_ps[:], RELU)

                    # hT via one batched DMA XBAR transpose, SBUF -> SBUF:
                    # ht_sb[e_in, ec, s] = h_se[s, ec*128 + e_in]
                    # All on the sync engine (the trigger occupies the
                    # issuing engine ~1.2us; sync does nothing else).
                    ht_sb = htp.tile([P, EC, P], bf16)
                    nc.sync.dma_start_transpose(ht_sb[:], h_se[:])

                    tiles.append((t, h_se, ht_sb))

            # ---- phase 2: scores -> exp -> U/Z accumulate ----
            p_tiles = {}
            for t, h_se_, ht_sb_ in tiles:
                # scores[s,h] = sum_e h[s,e] wkq[e,h]
                sc_ps = ps_s.tile([P, H], f32)
                for ec in range(EC):
                    nc.tensor.matmul(
                        sc_ps[:],
                        ht_sb_[:, ec, :],
                        wkq_sb[:, ec, :],
                        start=(ec == 0),
                        stop=(ec == EC - 1),
                    )
                # park scores in SBUF (DVE) so EXP reads cold data and the
                # scalar engine never gates a PSUM bank
                sc_sb = scp.tile([P, H], f32)
                nc.vector.tensor_copy(sc_sb[:], sc_ps[:])
                # p = exp(scores + maskbias); maskbias = 0 for unmasked,
                # -1e4 for masked (additive bias port, per-partition)
                p_sb = pp.tile([P, H], bf16)
                nc.scalar.activation(p_sb[:], sc_sb[:], EXP, bias=mb_t[:, t : t + 1])
                p_tiles[t] = p_sb
                # U/Z for tile t-3 (EXP pipeline depth 3)
                if t - 3 in p_tiles:
                    tu, h_u, _ = tiles[t - 3]
                    nc.tensor.matmul(
                        u_ps[:], p_tiles[tu][:], h_u[:],
                        start=(tu == 0), stop=(tu == S_TILES - 1),
                        skip_group_check=True,
                    )
                    nc.tensor.matmul(
                        z_ps[:], p_tiles[tu][:], ones_t[:],
                        start=(tu == 0), stop=(tu == S_TILES - 1),
                        skip_group_check=True,
                    )
            for t in range(S_TILES - 3, S_TILES):
                tu, h_u, _ = tiles[t]
                nc.tensor.matmul(
                    u_ps[:], p_tiles[tu][:], h_u[:],
                    start=(tu == 0), stop=(tu == S_TILES - 1),
                    skip_group_check=True,
                )
                nc.tensor.matmul(
                    z_ps[:], p_tiles[tu][:], ones_t[:],
                    start=(tu == 0), stop=(tu == S_TILES - 1),
                    skip_group_check=True,
                )

            u_sb = uzp.tile([H, E], f32, tag="u_sb")
            z_sb = uzp.tile([H, 1], f32, tag="z_sb")
            nc.vector.tensor_copy(u_sb[:], u_ps[:])
            nc.vector.tensor_copy(z_sb[:], z_ps[:, 0:1])
            nc.scalar.dma_start(u_out[b], u_sb[:])
            nc.scalar.dma_start(z_out[b], z_sb[:])

    nc.compile()
    with _nc_lock:
        _nc_cache[key] = nc
    return nc


WP_SCALE = 32.0  # Wp std ~0.031 -> ~1.0: keeps fp8e4 operands in normal range


def prepare_core_inputs(x, mask, Wp, wkq_scaled, bp=None):
    """Host-side packing for ONE core's shard.

    x: (BL, S, IN_DIM) fp32; mask: (BL, S) int; wkq_scaled: (E, H) fp32
    (already divided by WP_SCALE by the caller).
    """
    import ml_dtypes

    bf16 = ml_dtypes.bfloat16
    fp8 = ml_dtypes.float8_e4m3
    BL_, S_, I_ = x.shape
    IC = I_ // P
    EC = E // P
    SC = min(1024, S_)
    NCH = S_ // SC
    # xt[b, cc, q, p, c2, s] = x[b, cc*SC+s, (2q+c2)*128+p]: each (b,cc,q)
    # quarter is one contiguous [2, SC] run per partition p.
    xt = np.ascontiguousarray(
        x.reshape(BL_, NCH, SC, IC // 2, 2, P).transpose(0, 1, 3, 5, 4, 2)
    ).astype(fp8)
    wp = np.ascontiguousarray(Wp.reshape(IC, P, E) * WP_SCALE).astype(fp8)
    wkq = np.ascontiguousarray(wkq_scaled.reshape(EC, P, H)).astype(bf16)
    # additive mask bias packed [BL, P, S_TILES]: 0 where kept, -1e4 where
    # masked (exp(-1e4 + s) underflows to exactly 0)
    mb = np.ascontiguousarray(
        ((mask.astype(np.float32) - 1.0) * 1.0e4)
        .reshape(BL_, S_ // P, P)
        .transpose(0, 2, 1)
    ).astype(np.float32)
    m = {"xt": xt, "wp": wp, "wkq": wkq, "mb": mb}
    if bp is not None:
        m["bp"] = (np.asarray(bp) * WP_SCALE).astype(bf16).reshape(1, E)
    return m


def kernel(
    x, mask, query, Wp, bp, Wq, bq, Wk, bk, Wv, bv, Wo, bo, W2, b2, gamma, beta,
    _trace=False,
):
    x = np.asarray(x)
    mask = np.asarray(mask)
    BL = B // NCORES

    # Host-side folds (all tiny)
    qh = (np.asarray(query, np.float64) @ np.asarray(Wq, np.float64)
          + np.asarray(bq, np.float64)).reshape(H, D)
    # Device h' = WP_SCALE * h, so fold 1/WP_SCALE into the score weights.
    wkq_scaled = np.einsum(
        "ehd,hd->eh",
        np.asarray(Wk, np.float64).reshape(E, H, D),
        qh,
    ) / (np.sqrt(D) * WP_SCALE)

    has_bp = bool(np.any(np.asarray(bp)))
    nc = build_nc(has_bp=has_bp)

    in_maps = []
    for c in range(NCORES):
        sl = slice(c * BL, (c + 1) * BL)
        in_maps.append(
            prepare_core_inputs(
                x[sl], mask[sl], np.asarray(Wp), wkq_scaled.astype(np.float32),
                bp=np.asarray(bp) if has_bp else None,
            )
        )

    res = run_bass_kernel_spmd(
        nc, in_maps, core_ids=list(range(NCORES)), trace=_trace
    )
    U = np.concatenate([r["u_out"] for r in res.results], axis=0)  # (B, H, E)
    Z = np.concatenate([r["z_out"] for r in res.results], axis=0)[..., :1]  # (B, H, 1)

    # Host epilogue in float64 (U is WP_SCALE x too big; fold into Wv)
    pooledH = U.astype(np.float64) / (Z.astype(np.float64) * WP_SCALE)  # (B, H, E)
    Wv64 = np.asarray(Wv, np.float64).reshape(E, H, D)
    pooled = np.einsum("bhe,ehd->bhd", pooledH, Wv64).reshape(B, E)
    pooled += np.asarray(bv, np.float64)
    pooled = pooled @ np.asarray(Wo, np.float64) + np.asarray(bo, np.float64)
    out = pooled @ np.asarray(W2, np.float64) + np.asarray(b2, np.float64)
    mu = out.mean(-1, keepdims=True)
    var = out.var(-1, keepdims=True)
    out = (out - mu) / np.sqrt(var + 1e-5) * np.asarray(gamma, np.float64) + np.asarray(
        beta, np.float64
    )
    out_f32 = out.astype(np.float32)
    if _trace:
        return out_f32, res
    return out_f32



# revision 40
# speedup vs baseline: 1.0533x; 1.0533x over previous
"""Trainium2 Bass kernel for nn_AttentionPoolingTemporalEncoder.

Strategy (data-parallel over batch, 8 cores, 4 batch rows each):
  device:  h = relu(x @ Wp)               (fp8e4 DoubleRow matmuls, 2x PE rate;
                                           Wp pre-scaled x32 into e4m3 range)
           scores = h @ ((Wk @ qh)/sqrt(D))   (bk shifts cancel in softmax)
           p = exp(scores + maskbias)     (no running max; scores are O(5))
           U[h,:] = sum_s p[s,h] * h[s,:] ; Z[h] = sum_s p[s,h]
  host:    pooled = (U/Z) @ Wv (+bv) per head; @Wo+bo; @W2+b2; LayerNorm.
"""

import sys
import threading

import numpy as np

sys.path.insert(0, "/opt/trn_rl_repo")

from contextlib import ExitStack

import concourse.tile as tile
from concourse import bacc, mybir
from concourse.bass_utils import run_bass_kernel_spmd
from concourse.masks import make_identity


def _ensure_axon_ntff_hook_module():
    """Some images lack ``antenv.axon_hooks``; concourse imports it
    unconditionally when tracing is requested (e.g. via BASS_TRACE).
    Provide a minimal stand-in so that path degrades to no-trace
    instead of crashing."""
    try:
        from antenv import axon_hooks  # noqa: F401

        return
    except ImportError:
        pass
    import types

    mod = types.ModuleType("antenv.axon_hooks")
    mod._hook = None

    def set_axon_ntff_profile_hook(h):
        mod._hook = h

    def get_axon_ntff_profile_hook():
        return mod._hook

    mod.set_axon_ntff_profile_hook = set_axon_ntff_profile_hook
    mod.get_axon_ntff_profile_hook = get_axon_ntff_profile_hook
    sys.modules["antenv.axon_hooks"] = mod
    try:
        import antenv

        antenv.axon_hooks = mod
    except ImportError:
        pass


_ensure_axon_ntff_hook_module()

# Problem sizes (hardcoded per spec)
B, S, IN_DIM, E, H = 32, 4096, 1024, 512, 8
D = E // H
NCORES = 8
P = 128

_nc_cache = {}
_nc_lock = threading.Lock()


def build_nc(BL=B // NCORES, S_=S, I_=IN_DIM, has_bp=False, no_mask=False, trace_label=""):
    """Build + compile the per-core Bass program.

    BL: batch rows per core. S_: sequence length. I_: input dim.
    has_bp: emit the extra K=1 matmul adding the input-projection bias.
    """
    key = (BL, S_, I_, has_bp, no_mask)
    with _nc_lock:
        if key in _nc_cache:
            return _nc_cache[key]

    IC = I_ // P        # input-dim chunks
    EC = E // P         # embed-dim chunks
    S_TILES = S_ // P   # sequence tiles per batch row
    S_BLK = min(1024, S_)
    BLKS = S_ // S_BLK
    TPB = S_BLK // P    # s-tiles per DMA block

    f32 = mybir.dt.float32
    bf16 = mybir.dt.bfloat16
    fp8 = mybir.dt.float8e4
    DR = mybir.MatmulPerfMode.DoubleRow
    RELU = mybir.ActivationFunctionType.Relu
    EXP = mybir.ActivationFunctionType.Exp
    COPY = mybir.ActivationFunctionType.Copy

    nc = bacc.Bacc(
        "TRN2",
        target_bir_lowering=False,
        debug=False,
        enable_asserts=False,
        num_devices=NCORES,
    )

    # DRAM I/O (per-core shapes). The big matmul operands are fp8e4
    # (host-cast): quarters HBM traffic for x and runs the PE at 2x rate
    # via DoubleRow (contraction pairs of 128-chunks).
    # x layout: [b, s_chunk, quarter, partition, 2, s] so each quarter-load
    # is one contiguous 2KB run per partition (cheap HWDGE descriptors).
    NCH_ = S_ // min(1024, S_)
    SC_ = min(1024, S_)
    xt = nc.dram_tensor(
        "xt", [BL, NCH_, IC // 2, P, 2, SC_], fp8, kind="ExternalInput"
    ).ap()
    wp = nc.dram_tensor("wp", [IC, P, E], fp8, kind="ExternalInput").ap()
    wkq = nc.dram_tensor("wkq", [EC, P, H], bf16, kind="ExternalInput").ap()
    mb = nc.dram_tensor("mb", [BL, P, S_TILES], f32, kind="ExternalInput").ap()
    if has_bp:
        bp_d = nc.dram_tensor("bp", [1, E], bf16, kind="ExternalInput").ap()
    u_out = nc.dram_tensor("u_out", [BL, H, E], f32, kind="ExternalOutput").ap()
    z_out = nc.dram_tensor("z_out", [BL, H, 1], f32, kind="ExternalOutput").ap()

    with tile.TileContext(nc) as tc, ExitStack() as ctx:
        const = ctx.enter_context(tc.tile_pool(name="const", bufs=1))
        xp = ctx.enter_context(tc.tile_pool(name="xp", bufs=4))
        # h_se and p_sb stay resident for a whole batch row (the U/Z
        # accumulation runs as an end-of-row burst), plus slack so the next
        # row can start while the burst drains.
        hp = ctx.enter_context(tc.tile_pool(name="hp", bufs=S_TILES + 7))
        htp = ctx.enter_context(tc.tile_pool(name="htp", bufs=S_TILES + 6))
        pp = ctx.enter_context(tc.tile_pool(name="pp", bufs=8))
        scp = ctx.enter_context(tc.tile_pool(name="scp", bufs=6))
        mbp = ctx.enter_context(tc.tile_pool(name="mbp", bufs=2))
        uzp = ctx.enter_context(tc.tile_pool(name="uzp", bufs=2))
        ps_h = ctx.enter_context(tc.tile_pool(name="ps_h", bufs=3, space="PSUM"))
        ps_s = ctx.enter_context(tc.tile_pool(name="ps_s", bufs=2, space="PSUM"))
        ps_u = ctx.enter_context(tc.tile_pool(name="ps_u", bufs=2, space="PSUM"))
        ps_z = ctx.enter_context(tc.tile_pool(name="ps_z", bufs=1, space="PSUM"))

        # Resident constants
        wp_sb = const.tile([P, IC, E], fp8)
        nc.sync.dma_start(wp_sb[:], wp.rearrange("c p e -> p c e"))
        wkq_sb = const.tile([P, EC, H], bf16)
        nc.sync.dma_start(wkq_sb[:], wkq.rearrange("c p h -> p c h"))
        ones_t = const.tile([P, 2], bf16)
        nc.gpsimd.memset(ones_t[:], 1.0)
        if has_bp:
            ones_row = const.tile([1, P], bf16)
            nc.gpsimd.memset(ones_row[:], 1.0)
            bp_sb = const.tile([1, E], bf16)
            nc.sync.dma_start(bp_sb[:], bp_d[:])

        # Chunked x prefetch (1024 s = 8 tiles per chunk), issued ahead so
        # loads never queue behind the per-tile transposes.
        SC = min(1024, S_)
        NCH = S_ // SC
        chunks = [(bb, cc) for bb in range(BL) for cc in range(NCH)]

        def alloc_chunk():
            xt_c = xp.tile([P, IC, SC], fp8, tag="xchunk")
            return xt_c

        def load_chunk_part(xt_c, idx, q, eng=None):
            # Quarter-chunk loads (2 of 8 i-chunks, 256 KB) keep DMA bursts
            # short so the latency-critical transposes are never stuck
            # behind megabyte transfers. The host layout makes each quarter
            # one contiguous 2KB run per partition, so the HWDGE trigger is
            # cheap on the issuing engine.
            bb, cc = chunks[idx]
            (eng or nc.scalar).dma_start(xt_c[:, 2 * q : 2 * q + 2, :], xt[bb, cc, q])

        def load_chunk(idx):
            xt_c = alloc_chunk()
            for q in range(4):
                load_chunk_part(xt_c, idx, q)
            return xt_c

        # distance-3 prefetch: three chunks in flight ahead of the consumer
        bufq = [load_chunk(i) for i in range(min(3, len(chunks)))]
        chunk_idx = len(bufq) - 1
        part_sched = []  # [(tile_countdown, xt_c, idx, q), ...]

        # Row-level software pipeline: row b's phase 1 (dense DoubleRow
        # h-MM stream + RELU + transpose, only x-prefetch deps) carries
        # row b-1's phase 2 (scores/EXP/U/Z) embedded per-tile. Every
        # phase-2 dependency is then a full row (~40us) old, so no engine
        # ever waits on freshly-produced data and the PE stream stays
        # dense enough to keep HAM at full clock.
        class Row:
            def __init__(row):
                row.tiles = []  # [(t, h_se, ht_sb), ...]
                row.p = {}  # t -> p_sb
                row.mb_t = None
                row.u_ps = None
                row.z_ps = None
                row.b = 0

        def p2_scores(row, t):
            # scores[s,h] = sum_e h[s,e] wkq[e,h]; park in SBUF via DVE
            _, h_se_, ht_sb_ = row.tiles[t]
            sc_ps = ps_s.tile([P, H], f32)
            for ec in range(EC):
                nc.tensor.matmul(
                    sc_ps[:],
                    ht_sb_[:, ec, :],
                    wkq_sb[:, ec, :],
                    start=(ec == 0),
                    stop=(ec == EC - 1),
                )
            sc_sb = scp.tile([P, H], f32)
            nc.vector.tensor_copy(sc_sb[:], sc_ps[:])
            return sc_sb

        def p2_exp(row, t, sc_sb):
            # p = exp(scores + maskbias); maskbias = 0 for unmasked, -1e4
            # for masked positions (additive bias port, per-partition).
            p_sb = pp.tile([P, H], bf16)
            nc.scalar.activation(p_sb[:], sc_sb[:], EXP, bias=row.mb_t[:, t : t + 1])
            row.p[t] = p_sb

        def p2_uz(row, t):
            if row.u_ps is None:
                row.u_ps = ps_u.tile([H, E], f32, name="u_ps")
                row.z_ps = ps_z.tile([H, 2], f32, name="z_ps")
            _, h_u, _ = row.tiles[t]
            nc.tensor.matmul(
                row.u_ps[:], row.p[t][:], h_u[:],
                start=(t == 0), stop=(t == S_TILES - 1),
                skip_group_check=True,
            )
            nc.tensor.matmul(
                row.z_ps[:], row.p[t][:], ones_t[:],
                start=(t == 0), stop=(t == S_TILES - 1),
                skip_group_check=True,
            )

        def p2_store(row):
            u_sb = uzp.tile([H, E], f32, tag="u_sb")
            z_sb = uzp.tile([H, 1], f32, tag="z_sb")
            nc.vector.tensor_copy(u_sb[:], row.u_ps[:])
            nc.vector.tensor_copy(z_sb[:], row.z_ps[:, 0:1])
            nc.scalar.dma_start(u_out[row.b], u_sb[:])
            nc.scalar.dma_start(z_out[row.b], z_sb[:])

        def p2_step(row, t):
            """Embedded phase-2 step for the previous row at host tile t.
            Stages: quad at t, EXP at t+4, U/Z at t+6 (with end flushes)."""
            if row is None:
                return
            if t < S_TILES:
                row._sc[t] = p2_scores(row, t)
            if 4 <= t and t - 4 < S_TILES:
                p2_exp(row, t - 4, row._sc.pop(t - 4))
            if 6 <= t and t - 6 < S_TILES:
                p2_uz(row, t - 6)
            if t == S_TILES + 5:
                p2_store(row)

        prev = None
        for b in range(BL):
            row = Row()
            row.b = b
            row._sc = {}
            row.mb_t = mbp.tile([P, S_TILES], f32, name="mb_t")
            nc.gpsimd.dma_start(row.mb_t[:], mb[b])

            # ---- phase 1 for row b, with row b-1's phase 2 embedded ----
            for t in range(S_TILES):
                    TPC = SC // P
                    if t % TPC == 0:
                        # consume the next chunk; schedule the +3 chunk's
                        # quarter-loads spread over this chunk's tiles
                        x_sb = bufq.pop(0)
                        if chunk_idx + 1 < len(chunks):
                            chunk_idx += 1
                            nxt = alloc_chunk()
                            bufq.append(nxt)
                            for q in range(4):
                                part_sched.append((2 * q, nxt, chunk_idx, q))
                    # fire due quarter-loads, age the rest
                    still = []
                    for cd, xc, idx, q in part_sched:
                        if cd <= 0:
                            load_chunk_part(xc, idx, q)
                        else:
                            still.append((cd - 1, xc, idx, q))
                    part_sched[:] = still
                    # previous row's phase-2 step (deps ~a full row old)
                    p2_step(prev, t)
                    # DoubleRow fp8: each MM contracts a pair of 128-deep
                    # i-chunks (256 total) at 2 fp8/cyc.
                    h_ps = ps_h.tile([P, E], f32)
                    for c in range(0, IC, 2):
                        nc.tensor.matmul(
                            h_ps[:],
                            x_sb[:, c : c + 2, (t % TPC) * P : (t % TPC + 1) * P],
                            wp_sb[:, c : c + 2, :],
                            start=(c == 0),
                            stop=(c == IC - 2) and not has_bp,
                            perf_mode=DR,
                        )
                    if has_bp:
                        nc.tensor.matmul(
                            h_ps[:],
                            ones_row[:],
                            bp_sb[:],
                            start=False,
                            stop=True,
                        )
                    h_se = hp.tile([P, E], bf16)
                    nc.scalar.activation(h_se[:], h_ps[:], RELU)

                    # hT via one batched DMA XBAR transpose, SBUF -> SBUF:
                    # ht_sb[e_in, ec, s] = h_se[s, ec*128 + e_in]
                    # All on the sync engine (the trigger occupies the
                    # issuing engine ~1.2us; sync does nothing else).
                    ht_sb = htp.tile([P, EC, P], bf16)
                    nc.sync.dma_start_transpose(ht_sb[:], h_se[:])

                    row.tiles.append((t, h_se, ht_sb))
            # finish the previous row's phase-2 tail
            if prev is not None:
                for t in range(S_TILES, S_TILES + 6):
                    p2_step(prev, t)
            prev = row

        # last row's phase 2 runs standalone (transposes lag ~8 tiles, so
        # stream it in the same staged order)
        for t in range(S_TILES + 6):
            p2_step(prev, t)

    nc.compile()
    with _nc_lock:
        _nc_cache[key] = nc
    return nc


WP_SCALE = 32.0  # Wp std ~0.031 -> ~1.0: keeps fp8e4 operands in normal range


def prepare_core_inputs(x, mask, Wp, wkq_scaled, bp=None):
    """Host-side packing for ONE core's shard.

    x: (BL, S, IN_DIM) fp32; mask: (BL, S) int; wkq_scaled: (E, H) fp32
    (already divided by WP_SCALE by the caller).
    """
    import ml_dtypes

    bf16 = ml_dtypes.bfloat16
    fp8 = ml_dtypes.float8_e4m3
    BL_, S_, I_ = x.shape
    IC = I_ // P
    EC = E // P
    SC = min(1024, S_)
    NCH = S_ // SC
    # xt[b, cc, q, p, c2, s] = x[b, cc*SC+s, (2q+c2)*128+p]: each (b,cc,q)
    # quarter is one contiguous [2, SC] run per partition p.
    xt = np.ascontiguousarray(
        x.reshape(BL_, NCH, SC, IC // 2, 2, P).transpose(0, 1, 3, 5, 4, 2)
    ).astype(fp8)
    wp = np.ascontiguousarray(Wp.reshape(IC, P, E) * WP_SCALE).astype(fp8)
    wkq = np.ascontiguousarray(wkq_scaled.reshape(EC, P, H)).astype(bf16)
    # additive mask bias packed [BL, P, S_TILES]: 0 where kept, -1e4 where
    # masked (exp(-1e4 + s) underflows to exactly 0)
    mb = np.ascontiguousarray(
        ((mask.astype(np.float32) - 1.0) * 1.0e4)
        .reshape(BL_, S_ // P, P)
        .transpose(0, 2, 1)
    ).astype(np.float32)
    m = {"xt": xt, "wp": wp, "wkq": wkq, "mb": mb}
    if bp is not None:
        m["bp"] = (np.asarray(bp) * WP_SCALE).astype(bf16).reshape(1, E)
    return m


def kernel(
    x, mask, query, Wp, bp, Wq, bq, Wk, bk, Wv, bv, Wo, bo, W2, b2, gamma, beta,
    _trace=False,
):
    x = np.asarray(x)
    mask = np.asarray(mask)
    BL = B // NCORES

    # Host-side folds (all tiny)
    qh = (np.asarray(query, np.float64) @ np.asarray(Wq, np.float64)
          + np.asarray(bq, np.float64)).reshape(H, D)
    # Device h' = WP_SCALE * h, so fold 1/WP_SCALE into the score weights.
    wkq_scaled = np.einsum(
        "ehd,hd->eh",
        np.asarray(Wk, np.float64).reshape(E, H, D),
        qh,
    ) / (np.sqrt(D) * WP_SCALE)

    has_bp = bool(np.any(np.asarray(bp)))
    nc = build_nc(has_bp=has_bp)

    in_maps = []
    for c in range(NCORES):
        sl = slice(c * BL, (c + 1) * BL)
        in_maps.append(
            prepare_core_inputs(
                x[sl], mask[sl], np.asarray(Wp), wkq_scaled.astype(np.float32),
                bp=np.asarray(bp) if has_bp else None,
            )
        )

    res = run_bass_kernel_spmd(
        nc, in_maps, core_ids=list(range(NCORES)), trace=_trace
    )
    U = np.concatenate([r["u_out"] for r in res.results], axis=0)  # (B, H, E)
    Z = np.concatenate([r["z_out"] for r in res.results], axis=0)[..., :1]  # (B, H, 1)

    # Host epilogue in float64 (U is WP_SCALE x too big; fold into Wv)
    pooledH = U.astype(np.float64) / (Z.astype(np.float64) * WP_SCALE)  # (B, H, E)
    Wv64 = np.asarray(Wv, np.float64).reshape(E, H, D)
    pooled = np.einsum("bhe,ehd->bhd", pooledH, Wv64).reshape(B, E)
    pooled += np.asarray(bv, np.float64)
    pooled = pooled @ np.asarray(Wo, np.float64) + np.asarray(bo, np.float64)
    out = pooled @ np.asarray(W2, np.float64) + np.asarray(b2, np.float64)
    mu = out.mean(-1, keepdims=True)
    var = out.var(-1, keepdims=True)
    out = (out - mu) / np.sqrt(var + 1e-5) * np.asarray(gamma, np.float64) + np.asarray(
        beta, np.float64
    )
    out_f32 = out.astype(np.float32)
    if _trace:
        return out_f32, res
    return out_f32

